# revision 12
# baseline (speedup 1.0000x reference)
"""CrossModalAttention Trainium2 kernel.

Per-core computation (data-parallel over batch, 1 sample per core):
  eeg_proj   = eeg @ W_e + b_e                  [T, U]
  image_proj = image @ W_i + b_i                [T, U]
  scores     = eeg_proj @ image_proj.T          [T, T]
  attn       = softmax(scores, axis=-1)
  att_eeg    = attn @ eeg_proj                  [T, U]
  att_img    = attn.T @ image_proj              [T, U]

Engine split: PE does only real matmuls (proj accumulation in f32r with
fp32 PSUM, scores + AV in fp16) plus the x-transposes (f32r, 1.5 c/row).
All other transposes run on the DMA XBAR (16x128 tile transpose):
projT->proj and the per-row-block E^T needed for att_eeg. Softmax max on
DVE, exp on ACT (exact per-row max subtraction), fp16 attention weights.
Phase 1 is software-pipelined per 512-row strip (image then eeg); phase 2
pipelines scores(qt) / softmax(qt) / E^T-XBAR(qt) / AV-eeg(qt-3).
Long-wait DMAs (XBARs, E^T) issue on SP; output stores on ACT.
End-to-end absmax relative error ~5e-3.
"""
import numpy as np
from contextlib import ExitStack

import concourse.bass as bass
import concourse.bacc as bacc
import concourse.mybir as mybir
import concourse.tile as tile
from concourse.bass_utils import run_bass_kernel_spmd
from concourse.masks import make_identity

F32 = mybir.dt.float32
F32R = mybir.dt.float32r
F16 = mybir.dt.float16
AX = mybir.AxisListType.X
EXP = mybir.ActivationFunctionType.Exp
COPY = mybir.ActivationFunctionType.Copy

B, T, DE, DI, U = 8, 2048, 512, 768, 256
NCORES = 8
TQ = T // 128           # 16 q/k tiles of 128
NK = T // 512           # 4 score chunks of 512
NSTRIP = T // 512       # 4 strips of 512 rows per modality
AV_LAG = 3              # AV-eeg trails scores by 3 q-tiles (XBAR latency)


def build():
    nc = bacc.Bacc("TRN2", target_bir_lowering=False, debug=False,
                   num_devices=NCORES)
    eeg = nc.dram_tensor("eeg", (T, DE), F32, kind="ExternalInput")
    image = nc.dram_tensor("image", (T, DI), F32, kind="ExternalInput")
    W_e = nc.dram_tensor("W_e", (DE, U), F32, kind="ExternalInput")
    b_e = nc.dram_tensor("b_e", (U,), F32, kind="ExternalInput")
    W_i = nc.dram_tensor("W_i", (DI, U), F32, kind="ExternalInput")
    b_i = nc.dram_tensor("b_i", (U,), F32, kind="ExternalInput")
    att_e = nc.dram_tensor("att_e", (T, U), F32, kind="ExternalOutput")
    att_i = nc.dram_tensor("att_i", (T, U), F32, kind="ExternalOutput")

    with ExitStack() as ctx:
        tc = ctx.enter_context(tile.TileContext(nc))
        const = ctx.enter_context(tc.tile_pool(name="const", bufs=1))
        persist = ctx.enter_context(tc.tile_pool(name="persist", bufs=1))
        xstrip = ctx.enter_context(tc.tile_pool(name="xstrip", bufs=2))
        xt = ctx.enter_context(tc.tile_pool(name="xt", bufs=6))
        ps = ctx.enter_context(tc.tile_pool(name="ps", bufs=6, space="PSUM"))
        psb = ctx.enter_context(tc.tile_pool(name="psb", bufs=2, space="PSUM"))
        small = ctx.enter_context(tc.tile_pool(name="small", bufs=4))
        etp = ctx.enter_context(tc.tile_pool(name="etp", bufs=4))
        outp = ctx.enter_context(tc.tile_pool(name="outp", bufs=2))

        ident = const.tile([128, 128], F32)
        make_identity(nc, ident[:])
        identr = const.tile([128, 128], F32R)
        nc.vector.tensor_copy(identr[:], ident[:])

        w_i_sb = const.tile([128, DI // 128, U], F32R)
        w_e_sb = const.tile([128, DE // 128, U], F32R)
        be_col = const.tile([128, 2], F32)
        bi_col = const.tile([128, 2], F32)

        projTe = persist.tile([128, 2, T], F16, tag="projTe")
        projTi = persist.tile([128, 2, T], F16, tag="projTi")
        # proj layout [128, uc, kc, 128]: XBAR writes disjoint [:, uc] slices
        proj_e = persist.tile([128, 2, TQ, 128], F16, tag="proj_e")
        proj_i = persist.tile([128, 2, TQ, 128], F16, tag="proj_i")
        E = persist.tile([128, TQ, T], F16, tag="E")
        rZ = persist.tile([128, TQ], F32, tag="rZ")

        # ---- biases first (tiny), then strip stream with W interleaved ----
        nc.sync.dma_start(out=bi_col[:],
                          in_=b_i.ap().rearrange("(c p) -> p c", p=128))
        nc.sync.dma_start(out=be_col[:],
                          in_=b_e.ap().rearrange("(c p) -> p c", p=128))

        MODS = [
            dict(x=image, D=DI, w=w_i_sb, b=bi_col, projT=projTi),
            dict(x=eeg, D=DE, w=w_e_sb, b=be_col, projT=projTe),
        ]
        for m in MODS:
            m["tiles"] = [
                xt.tile([128, T], F32R, tag="xt",
                        name=f"xT_{m['x'].name}_{dc}", uniquify=True)
                for dc in range(m["D"] // 128)]

        def emit_load(mi, s):
            m = MODS[mi]
            D = m["D"]
            x = m["x"].ap().bitcast(F32R)
            r0 = s * 512
            xs = xstrip.tile([128, 4, DI], F32R, tag="xs",
                             name=f"xs_{m['x'].name}_{s}")
            m.setdefault("xs", {})[s] = xs
            if (mi, s) == (0, 0):
                for h in range(2):
                    nc.sync.dma_start(
                        out=xs[:, 2 * h:2 * h + 2, :D],
                        in_=x[r0 + h * 256:r0 + (h + 1) * 256, :].rearrange(
                            "(tt p) d -> p tt d", p=128))
            else:
                nc.sync.dma_start(
                    out=xs[:, :, :D],
                    in_=x[r0:r0 + 512, :].rearrange(
                        "(tt p) d -> p tt d", p=128))
            # W loads interleaved into the strip stream right after the
            # strip that precedes first use
            if (mi, s) == (0, 1):
                nc.sync.dma_start(
                    out=w_i_sb[:],
                    in_=W_i.ap().bitcast(F32R).rearrange(
                        "(c p) u -> p c u", p=128))
            if (mi, s) == (1, 0):
                nc.sync.dma_start(
                    out=w_e_sb[:],
                    in_=W_e.ap().bitcast(F32R).rearrange(
                        "(c p) u -> p c u", p=128))

        def emit_xp(mi, s):
            m = MODS[mi]
            D = m["D"]
            r0 = s * 512
            xs = m["xs"][s]
            for dc in range(D // 128):
                pst = ps.tile([128, 512], F32R, tag="ps")
                for tt in range(4):
                    nc.tensor.transpose(
                        pst[:, tt * 128:(tt + 1) * 128],
                        xs[:, tt, dc * 128:(dc + 1) * 128], identr)
                nc.vector.tensor_copy(m["tiles"][dc][:, r0:r0 + 512], pst[:])

        def emit_proj(mi, s):
            m = MODS[mi]
            D = m["D"]
            r0 = s * 512
            for uc in range(2):
                pp = psb.tile([128, 512], F32, tag="psb")
                for dc in range(D // 128):
                    nc.tensor.matmul(
                        pp[:], m["w"][:, dc, uc * 128:(uc + 1) * 128],
                        m["tiles"][dc][:, r0:r0 + 512],
                        start=(dc == 0), stop=(dc == D // 128 - 1))
                nc.scalar.add(m["projT"][:, uc, r0:r0 + 512], pp[:],
                              add=m["b"][:, uc:uc + 1])

        # phase 1 pipeline: image strips 0..3 then eeg strips 0..3;
        # step k: load(k) + transpose(k), then projection(k-1)
        strips = [(0, s) for s in range(NSTRIP)] + \
                 [(1, s) for s in range(NSTRIP)]
        for k, (mi, s) in enumerate(strips):
            emit_load(mi, s)
            emit_xp(mi, s)
            if k > 0:
                emit_proj(*strips[k - 1])
        emit_proj(*strips[-1])

        # projT -> proj XBARs (SP queue; image's can run once projTi done)
        for uc in range(2):
            nc.sync.dma_start_transpose(proj_i[:, uc], projTi[:, uc, :])

        # ---- phase 2 ----
        ets = [None] * TQ
        finz = [None] * TQ

        def emit_scores(qt):
            cm = small.tile([128, 4], F32, tag="cm", name=f"cm_{qt}")
            s_chunks = []
            for nk in range(NK):
                s = ps.tile([128, 512], F32, tag="ps", name=f"s_{qt}_{nk}")
                s_chunks.append(s)
                for uc in range(2):
                    nc.tensor.matmul(
                        s[:],
                        projTe[:, uc, qt * 128:(qt + 1) * 128],
                        projTi[:, uc, nk * 512:(nk + 1) * 512],
                        start=(uc == 0), stop=(uc == 1))
                nc.vector.reduce_max(cm[:, nk:nk + 1], s[:], axis=AX)
            negmax = small.tile([128, 1], F32, tag="negmax", name=f"nm_{qt}")
            nc.vector.tensor_reduce(negmax[:], cm[:], axis=AX,
                                    op=mybir.AluOpType.max, negate=True)
            zp = small.tile([128, 4], F32, tag="zp", name=f"zp_{qt}")
            for nk in range(NK):
                nc.scalar.activation(
                    E[:, qt, nk * 512:(nk + 1) * 512], s_chunks[nk][:], EXP,
                    bias=negmax[:], scale=1.0, accum_out=zp[:, nk:nk + 1])
            ett = etp.tile([128, TQ, 128], F16, tag="ett", name=f"ett_{qt}")
            nc.sync.dma_start_transpose(ett[:], E[:, qt, :])
            ets[qt] = ett
            finz[qt] = zp

        def emit_finalize_z(qt):
            # deferred one iteration so DVE's wait-queue isn't clogged in
            # front of the next qt's reduce_max chain
            zp = finz[qt]
            zrow = small.tile([128, 1], F32, tag="zrow", name=f"zr_{qt}")
            nc.vector.reduce_sum(zrow[:], zp[:], axis=AX)
            nc.vector.reciprocal(rZ[:, qt:qt + 1], zrow[:])
            nc.vector.tensor_scalar_mul(
                proj_i[:, :, qt, :], proj_i[:, :, qt, :], rZ[:, qt:qt + 1])

        oe_buf = [None]

        def emit_av_eeg(qt):
            ett = ets[qt]
            pav = psb.tile([128, 512], F32, tag="psb", name=f"pav_{qt}")
            for kc in range(TQ):
                nc.tensor.matmul(pav[:, :U], ett[:, kc, :],
                                 proj_e[:, :, kc, :],
                                 start=(kc == 0), stop=(kc == TQ - 1))
            ets[qt] = None
            if qt % 2 == 0:
                oe_buf[0] = outp.tile([128, 2, U], F32, tag="oute",
                                      name=f"oe_{qt}")
            nc.scalar.activation(oe_buf[0][:, qt % 2, :], pav[:, :U], COPY,
                                 scale=rZ[:, qt:qt + 1])
            if qt % 2 == 1:
                q0 = (qt - 1) * 128
                nc.scalar.dma_start(
                    out=att_e.ap()[q0:q0 + 256, :].rearrange(
                        "(c p) u -> p c u", p=128),
                    in_=oe_buf[0][:])

        for qt in range(TQ):
            emit_scores(qt)
            if qt == 0:
                # eeg projT->proj XBARs: issued after ETT(0) on SP so ETT(0)
                # is not delayed behind their projTe wait
                for uc in range(2):
                    nc.sync.dma_start_transpose(proj_e[:, uc],
                                                projTe[:, uc, :])
            if qt >= 1:
                emit_finalize_z(qt - 1)
            if qt >= AV_LAG:
                emit_av_eeg(qt - AV_LAG)
        emit_finalize_z(TQ - 1)
        for qt in range(TQ - AV_LAG, TQ):
            emit_av_eeg(qt)

        # ---- phase 3: att_img[kt] = sum_q E[q, kt-block].T @ (proj_i/Z)[q]
        oi_buf = None
        for kt in range(TQ):
            pav = psb.tile([128, 512], F32, tag="psb", name=f"pvi_{kt}")
            for qc in range(TQ):
                nc.tensor.matmul(
                    pav[:, :U], E[:, qc, kt * 128:(kt + 1) * 128],
                    proj_i[:, :, qc, :],
                    start=(qc == 0), stop=(qc == TQ - 1))
            if kt % 2 == 0:
                oi_buf = outp.tile([128, 2, U], F32, tag="outi",
                                   name=f"oi_{kt}")
            nc.scalar.copy(oi_buf[:, kt % 2, :], pav[:, :U])
            if kt % 2 == 1:
                k0 = (kt - 1) * 128
                nc.scalar.dma_start(
                    out=att_i.ap()[k0:k0 + 256, :].rearrange(
                        "(c p) u -> p c u", p=128),
                    in_=oi_buf[:])

    nc.finalize()
    return nc


_NC_CACHE = {}


def kernel(eeg, image, W_e, b_e, W_i, b_i):
    key = "v3"
    if key not in _NC_CACHE:
        _NC_CACHE[key] = build()
    nc = _NC_CACHE[key]
    eeg = np.ascontiguousarray(eeg, dtype=np.float32)
    image = np.ascontiguousarray(image, dtype=np.float32)
    in_maps = [{
        "eeg": eeg[b], "image": image[b],
        "W_e": np.asarray(W_e, np.float32), "b_e": np.asarray(b_e, np.float32),
        "W_i": np.asarray(W_i, np.float32), "b_i": np.asarray(b_i, np.float32),
    } for b in range(B)]
    res = run_bass_kernel_spmd(nc, in_maps, list(range(NCORES)))
    att_e = np.stack([np.asarray(r["att_e"]) for r in res.results])
    att_i = np.stack([np.asarray(r["att_i"]) for r in res.results])
    return att_e, att_i


# revision 15
# speedup vs baseline: 1.0770x; 1.0770x over previous
"""CrossModalAttention Trainium2 kernel.

Per-core computation (data-parallel over batch, 1 sample per core):
  eeg_proj   = eeg @ W_e + b_e                  [T, U]
  image_proj = image @ W_i + b_i                [T, U]
  scores     = eeg_proj @ image_proj.T          [T, T]
  attn       = softmax(scores, axis=-1)
  att_eeg    = attn @ eeg_proj                  [T, U]
  att_img    = attn.T @ image_proj              [T, U]

Engine split: PE does only real matmuls (proj accumulation in f32r with
fp32 PSUM, scores + AV in fp16) plus the x-transposes (f32r, 1.5 c/row).
All other transposes run on the DMA XBAR (16x128 tile transpose):
projT->proj and the per-row-block E^T needed for att_eeg. Softmax row
max on DVE, exp on ACT (exact per-row max subtraction), Z-finalization
(sum + proj_i prescale) on the otherwise-idle Pool engine, fp16
attention weights. Phase 1 is software-pipelined per 256-row strip
(image then eeg); phase 2 pipelines scores(qt) / softmax(qt) /
E^T-XBAR(qt) / AV-eeg(qt-4). Long-wait DMAs (XBARs, E^T) issue on SP;
output stores on ACT. End-to-end absmax relative error ~5e-3.
"""
import numpy as np
from contextlib import ExitStack

import concourse.bass as bass
import concourse.bacc as bacc
import concourse.mybir as mybir
import concourse.tile as tile
from concourse.bass_utils import run_bass_kernel_spmd
from concourse.masks import make_identity

F32 = mybir.dt.float32
F32R = mybir.dt.float32r
F16 = mybir.dt.float16
AX = mybir.AxisListType.X
EXP = mybir.ActivationFunctionType.Exp
COPY = mybir.ActivationFunctionType.Copy

B, T, DE, DI, U = 8, 2048, 512, 768, 256
NCORES = 8
TQ = T // 128           # 16 q/k tiles of 128
NK = T // 512           # 4 score chunks of 512
NPAIR = T // 512        # 4 strip-pairs (of 2x256 rows) per modality
AV_LAG = 4              # AV-eeg trails scores by 4 q-tiles (XBAR latency)


def build():
    nc = bacc.Bacc("TRN2", target_bir_lowering=False, debug=False,
                   num_devices=NCORES)
    eeg = nc.dram_tensor("eeg", (T, DE), F32, kind="ExternalInput")
    image = nc.dram_tensor("image", (T, DI), F32, kind="ExternalInput")
    W_e = nc.dram_tensor("W_e", (DE, U), F32, kind="ExternalInput")
    b_e = nc.dram_tensor("b_e", (U,), F32, kind="ExternalInput")
    W_i = nc.dram_tensor("W_i", (DI, U), F32, kind="ExternalInput")
    b_i = nc.dram_tensor("b_i", (U,), F32, kind="ExternalInput")
    att_e = nc.dram_tensor("att_e", (T, U), F32, kind="ExternalOutput")
    att_i = nc.dram_tensor("att_i", (T, U), F32, kind="ExternalOutput")

    with ExitStack() as ctx:
        tc = ctx.enter_context(tile.TileContext(nc))
        const = ctx.enter_context(tc.tile_pool(name="const", bufs=1))
        persist = ctx.enter_context(tc.tile_pool(name="persist", bufs=1))
        xstrip = ctx.enter_context(tc.tile_pool(name="xstrip", bufs=3))
        xt = ctx.enter_context(tc.tile_pool(name="xt", bufs=6))
        ps = ctx.enter_context(tc.tile_pool(name="ps", bufs=6, space="PSUM"))
        psb = ctx.enter_context(tc.tile_pool(name="psb", bufs=2, space="PSUM"))
        small = ctx.enter_context(tc.tile_pool(name="small", bufs=4))
        etp = ctx.enter_context(tc.tile_pool(name="etp", bufs=5))
        outp = ctx.enter_context(tc.tile_pool(name="outp", bufs=3))

        ident = const.tile([128, 128], F32)
        make_identity(nc, ident[:])
        identr = const.tile([128, 128], F32R)
        nc.vector.tensor_copy(identr[:], ident[:])

        w_i_sb = const.tile([128, DI // 128, U], F32R)
        w_e_sb = const.tile([128, DE // 128, U], F32R)
        be_col = const.tile([128, 2], F32)
        bi_col = const.tile([128, 2], F32)

        projTe = persist.tile([128, 2, T], F16, tag="projTe")
        projTi = persist.tile([128, 2, T], F16, tag="projTi")
        # proj layout [128, uc, kc, 128]: XBAR writes disjoint [:, uc] slices
        proj_e = persist.tile([128, 2, TQ, 128], F16, tag="proj_e")
        proj_i = persist.tile([128, 2, TQ, 128], F16, tag="proj_i")
        E = persist.tile([128, TQ, T], F16, tag="E")
        rZ = persist.tile([128, TQ], F32, tag="rZ")

        # ---- biases first (tiny) ----
        nc.sync.dma_start(out=bi_col[:],
                          in_=b_i.ap().rearrange("(c p) -> p c", p=128))
        nc.sync.dma_start(out=be_col[:],
                          in_=b_e.ap().rearrange("(c p) -> p c", p=128))

        MODS = [
            dict(x=image, D=DI, w=w_i_sb, b=bi_col, projT=projTi),
            dict(x=eeg, D=DE, w=w_e_sb, b=be_col, projT=projTe),
        ]
        for m in MODS:
            m["tiles"] = [
                xt.tile([128, T], F32R, tag="xt",
                        name=f"xT_{m['x'].name}_{dc}", uniquify=True)
                for dc in range(m["D"] // 128)]
            m["pst"] = {}

        def emit_load(mi, s):
            # s: 256-row strip index (0..7)
            m = MODS[mi]
            D = m["D"]
            x = m["x"].ap().bitcast(F32R)
            r0 = s * 256
            xs = xstrip.tile([128, 2, DI], F32R, tag="xs",
                             name=f"xs_{m['x'].name}_{s}")
            m.setdefault("xs", {})[s] = xs
            nc.sync.dma_start(
                out=xs[:, :, :D],
                in_=x[r0:r0 + 256, :].rearrange("(tt p) d -> p tt d", p=128))
            # W loads interleaved into the strip stream before first use
            if (mi, s) == (0, 2):
                nc.sync.dma_start(
                    out=w_i_sb[:],
                    in_=W_i.ap().bitcast(F32R).rearrange(
                        "(c p) u -> p c u", p=128))
            if (mi, s) == (1, 1):
                nc.sync.dma_start(
                    out=w_e_sb[:],
                    in_=W_e.ap().bitcast(F32R).rearrange(
                        "(c p) u -> p c u", p=128))

        def emit_xp(mi, s):
            # transpose strip s into the psum pair tile; copy out on odd s
            m = MODS[mi]
            D = m["D"]
            xs = m["xs"].pop(s)
            half = (s % 2) * 256
            for dc in range(D // 128):
                if s % 2 == 0:
                    m["pst"][dc] = ps.tile([128, 512], F32R, tag="ps",
                                           name=f"pst_{mi}_{s}_{dc}")
                pst = m["pst"][dc]
                for tt in range(2):
                    nc.tensor.transpose(
                        pst[:, half + tt * 128:half + (tt + 1) * 128],
                        xs[:, tt, dc * 128:(dc + 1) * 128], identr)
            if s % 2 == 1:
                p0 = (s // 2) * 512
                for dc in range(D // 128):
                    nc.vector.tensor_copy(
                        m["tiles"][dc][:, p0:p0 + 512], m["pst"][dc][:])

        def emit_proj(mi, p):
            m = MODS[mi]
            D = m["D"]
            r0 = p * 512
            for uc in range(2):
                pp = psb.tile([128, 512], F32, tag="psb")
                for dc in range(D // 128):
                    nc.tensor.matmul(
                        pp[:], m["w"][:, dc, uc * 128:(uc + 1) * 128],
                        m["tiles"][dc][:, r0:r0 + 512],
                        start=(dc == 0), stop=(dc == D // 128 - 1))
                nc.scalar.add(m["projT"][:, uc, r0:r0 + 512], pp[:],
                              add=m["b"][:, uc:uc + 1])

        # phase 1 pipeline over strip-pairs: image pairs 0..3, eeg pairs 0..3
        pairs = [(0, p) for p in range(NPAIR)] + \
                [(1, p) for p in range(NPAIR)]
        for k, (mi, p) in enumerate(pairs):
            emit_load(mi, 2 * p)
            emit_xp(mi, 2 * p)
            emit_load(mi, 2 * p + 1)
            emit_xp(mi, 2 * p + 1)
            if k > 0:
                emit_proj(*pairs[k - 1])
        emit_proj(*pairs[-1])

        # projT -> proj XBARs (SP queue)
        for uc in range(2):
            nc.sync.dma_start_transpose(proj_i[:, uc], projTi[:, uc, :])
        for uc in range(2):
            nc.sync.dma_start_transpose(proj_e[:, uc], projTe[:, uc, :])

        # ---- phase 2 ----
        ets = [None] * TQ
        finz = [None] * TQ

        def emit_scores(qt):
            cm = small.tile([128, 4], F32, tag="cm", name=f"cm_{qt}")
            s_chunks = []
            for nk in range(NK):
                s = ps.tile([128, 512], F32, tag="ps", name=f"s_{qt}_{nk}")
                s_chunks.append(s)
                for uc in range(2):
                    nc.tensor.matmul(
                        s[:],
                        projTe[:, uc, qt * 128:(qt + 1) * 128],
                        projTi[:, uc, nk * 512:(nk + 1) * 512],
                        start=(uc == 0), stop=(uc == 1))
                nc.vector.reduce_max(cm[:, nk:nk + 1], s[:], axis=AX)
            negmax = small.tile([128, 1], F32, tag="negmax", name=f"nm_{qt}")
            nc.vector.tensor_reduce(negmax[:], cm[:], axis=AX,
                                    op=mybir.AluOpType.max, negate=True)
            zp = small.tile([128, 4], F32, tag="zp", name=f"zp_{qt}")
            for nk in range(NK):
                nc.scalar.activation(
                    E[:, qt, nk * 512:(nk + 1) * 512], s_chunks[nk][:], EXP,
                    bias=negmax[:], scale=1.0, accum_out=zp[:, nk:nk + 1])
            ett = etp.tile([128, TQ, 128], F16, tag="ett", name=f"ett_{qt}")
            nc.sync.dma_start_transpose(ett[:], E[:, qt, :])
            ets[qt] = ett
            finz[qt] = zp

        def emit_finalize_z(qt):
            # on Pool (idle engine): Z row-sum and proj_i prescale;
            # reciprocal on DVE (no gpsimd reciprocal)
            zp = finz[qt]
            zh = small.tile([128, 2], F32, tag="zh", name=f"zh_{qt}")
            zrow = small.tile([128, 1], F32, tag="zrow", name=f"zr_{qt}")
            nc.gpsimd.tensor_add(zh[:], zp[:, 0:2], zp[:, 2:4])
            nc.gpsimd.tensor_add(zrow[:], zh[:, 0:1], zh[:, 1:2])
            nc.vector.reciprocal(rZ[:, qt:qt + 1], zrow[:])
            nc.gpsimd.tensor_scalar_mul(
                proj_i[:, :, qt, :], proj_i[:, :, qt, :], rZ[:, qt:qt + 1])

        oe_buf = [None]

        def emit_av_eeg(qt):
            ett = ets[qt]
            pav = psb.tile([128, 512], F32, tag="psb", name=f"pav_{qt}")
            for kc in range(TQ):
                nc.tensor.matmul(pav[:, :U], ett[:, kc, :],
                                 proj_e[:, :, kc, :],
                                 start=(kc == 0), stop=(kc == TQ - 1))
            ets[qt] = None
            if qt % 2 == 0:
                oe_buf[0] = outp.tile([128, 2, U], F32, tag="oute",
                                      name=f"oe_{qt}")
            nc.scalar.activation(oe_buf[0][:, qt % 2, :], pav[:, :U], COPY,
                                 scale=rZ[:, qt:qt + 1])
            if qt % 2 == 1:
                q0 = (qt - 1) * 128
                nc.scalar.dma_start(
                    out=att_e.ap()[q0:q0 + 256, :].rearrange(
                        "(c p) u -> p c u", p=128),
                    in_=oe_buf[0][:])

        for qt in range(TQ):
            emit_scores(qt)
            if qt >= 1:
                emit_finalize_z(qt - 1)
            if qt >= AV_LAG:
                emit_av_eeg(qt - AV_LAG)
        emit_finalize_z(TQ - 1)
        for qt in range(TQ - AV_LAG, TQ):
            emit_av_eeg(qt)

        # ---- phase 3: att_img[kt] = sum_q E[q, kt-block].T @ (proj_i/Z)[q]
        oi_buf = None
        for kt in range(TQ):
            pav = psb.tile([128, 512], F32, tag="psb", name=f"pvi_{kt}")
            for qc in range(TQ):
                nc.tensor.matmul(
                    pav[:, :U], E[:, qc, kt * 128:(kt + 1) * 128],
                    proj_i[:, :, qc, :],
                    start=(qc == 0), stop=(qc == TQ - 1))
            if kt % 2 == 0:
                oi_buf = outp.tile([128, 2, U], F32, tag="outi",
                                   name=f"oi_{kt}")
            nc.scalar.copy(oi_buf[:, kt % 2, :], pav[:, :U])
            if kt % 2 == 1:
                k0 = (kt - 1) * 128
                nc.scalar.dma_start(
                    out=att_i.ap()[k0:k0 + 256, :].rearrange(
                        "(c p) u -> p c u", p=128),
                    in_=oi_buf[:])

    nc.finalize()
    return nc


_NC_CACHE = {}


def kernel(eeg, image, W_e, b_e, W_i, b_i):
    key = "v4"
    if key not in _NC_CACHE:
        _NC_CACHE[key] = build()
    nc = _NC_CACHE[key]
    eeg = np.ascontiguousarray(eeg, dtype=np.float32)
    image = np.ascontiguousarray(image, dtype=np.float32)
    in_maps = [{
        "eeg": eeg[b], "image": image[b],
        "W_e": np.asarray(W_e, np.float32), "b_e": np.asarray(b_e, np.float32),
        "W_i": np.asarray(W_i, np.float32), "b_i": np.asarray(b_i, np.float32),
    } for b in range(B)]
    res = run_bass_kernel_spmd(nc, in_maps, list(range(NCORES)))
    att_e = np.stack([np.asarray(r["att_e"]) for r in res.results])
    att_i = np.stack([np.asarray(r["att_i"]) for r in res.results])
    return att_e, att_i


# revision 20
# speedup vs baseline: 1.0979x; 1.0194x over previous
"""CrossModalAttention Trainium2 kernel.

Per-core computation (data-parallel over batch, 1 sample per core):
  eeg_proj   = eeg @ W_e + b_e                  [T, U]
  image_proj = image @ W_i + b_i                [T, U]
  scores     = eeg_proj @ image_proj.T          [T, T]
  attn       = softmax(scores, axis=-1)
  att_eeg    = attn @ eeg_proj                  [T, U]
  att_img    = attn.T @ image_proj              [T, U]

Engine split: PE does only real matmuls (proj accumulation in f32r with
fp32 PSUM, scores + AV in fp16) plus the x-transposes (f32r, 1.5 c/row).
All other transposes run on the DMA XBAR (16x128 tile transpose):
projT->proj and the per-row-block E^T needed for att_eeg. Softmax row
max on DVE, exp on ACT (exact per-row max subtraction), Z-finalization
(sum + proj_i prescale) on the otherwise-idle Pool engine, fp16
attention weights. Phase 1 is software-pipelined per 256-row strip
(image then eeg); phase 2 pipelines scores(qt) / softmax(qt) /
E^T-XBAR(qt) / AV-eeg(qt-4). Long-wait DMAs (XBARs, E^T) issue on SP;
output stores on ACT. End-to-end absmax relative error ~5e-3.
"""
import numpy as np
from contextlib import ExitStack

import concourse.bass as bass
import concourse.bacc as bacc
import concourse.mybir as mybir
import concourse.tile as tile
from concourse.bass_utils import run_bass_kernel_spmd
from concourse.masks import make_identity

F32 = mybir.dt.float32
F32R = mybir.dt.float32r
F16 = mybir.dt.float16
AX = mybir.AxisListType.X
EXP = mybir.ActivationFunctionType.Exp
COPY = mybir.ActivationFunctionType.Copy

B, T, DE, DI, U = 8, 2048, 512, 768, 256
NCORES = 8
TQ = T // 128           # 16 q/k tiles of 128
NK = T // 512           # 4 score chunks of 512
NPAIR = T // 512        # 4 strip-pairs (of 2x256 rows) per modality
AV_LAG = 4              # AV-eeg trails scores by 4 q-tiles (XBAR latency)


def build():
    nc = bacc.Bacc("TRN2", target_bir_lowering=False, debug=False,
                   num_devices=NCORES)
    eeg = nc.dram_tensor("eeg", (T, DE), F32, kind="ExternalInput")
    image = nc.dram_tensor("image", (T, DI), F32, kind="ExternalInput")
    W_e = nc.dram_tensor("W_e", (DE, U), F32, kind="ExternalInput")
    b_e = nc.dram_tensor("b_e", (U,), F32, kind="ExternalInput")
    W_i = nc.dram_tensor("W_i", (DI, U), F32, kind="ExternalInput")
    b_i = nc.dram_tensor("b_i", (U,), F32, kind="ExternalInput")
    att_e = nc.dram_tensor("att_e", (T, U), F32, kind="ExternalOutput")
    att_i = nc.dram_tensor("att_i", (T, U), F32, kind="ExternalOutput")

    with ExitStack() as ctx:
        tc = ctx.enter_context(tile.TileContext(nc))
        const = ctx.enter_context(tc.tile_pool(name="const", bufs=1))
        persist = ctx.enter_context(tc.tile_pool(name="persist", bufs=1))
        xstrip = ctx.enter_context(tc.tile_pool(name="xstrip", bufs=3))
        xt = ctx.enter_context(tc.tile_pool(name="xt", bufs=6))
        ps = ctx.enter_context(tc.tile_pool(name="ps", bufs=6, space="PSUM"))
        psb = ctx.enter_context(tc.tile_pool(name="psb", bufs=2, space="PSUM"))
        small = ctx.enter_context(tc.tile_pool(name="small", bufs=4))
        etp = ctx.enter_context(tc.tile_pool(name="etp", bufs=5))
        outp = ctx.enter_context(tc.tile_pool(name="outp", bufs=3))

        ident = const.tile([128, 128], F32)
        make_identity(nc, ident[:])
        identr = const.tile([128, 128], F32R)
        nc.vector.tensor_copy(identr[:], ident[:])

        w_i_sb = const.tile([128, DI // 128, U], F32R)
        w_e_sb = const.tile([128, DE // 128, U], F32R)
        be_col = const.tile([128, 2], F32)
        bi_col = const.tile([128, 2], F32)

        projTe = persist.tile([128, 2, T], F16, tag="projTe")
        projTi = persist.tile([128, 2, T], F16, tag="projTi")
        # proj layout [128, uc, kc, 128]: XBAR writes disjoint [:, uc] slices
        proj_e = persist.tile([128, 2, TQ, 128], F16, tag="proj_e")
        proj_i = persist.tile([128, 2, TQ, 128], F16, tag="proj_i")
        E = persist.tile([128, TQ, T], F16, tag="E")
        rZ = persist.tile([128, TQ], F32, tag="rZ")

        MODS = [
            dict(x=image, D=DI, w=w_i_sb, b=bi_col, projT=projTi),
            dict(x=eeg, D=DE, w=w_e_sb, b=be_col, projT=projTe),
        ]
        for m in MODS:
            m["tiles"] = [
                xt.tile([128, T], F32R, tag="xt",
                        name=f"xT_{m['x'].name}_{dc}", uniquify=True)
                for dc in range(m["D"] // 128)]
            m["pst"] = {}

        def emit_load(mi, s):
            # s: 256-row strip index (0..7)
            m = MODS[mi]
            D = m["D"]
            x = m["x"].ap().bitcast(F32R)
            r0 = s * 256
            xs = xstrip.tile([128, 2, DI], F32R, tag="xs",
                             name=f"xs_{m['x'].name}_{s}")
            m.setdefault("xs", {})[s] = xs
            nc.sync.dma_start(
                out=xs[:, :, :D],
                in_=x[r0:r0 + 256, :].rearrange("(tt p) d -> p tt d", p=128))
            # W/b loads interleaved into the strip stream before first use
            if (mi, s) == (0, 2):
                nc.sync.dma_start(
                    out=w_i_sb[:],
                    in_=W_i.ap().bitcast(F32R).rearrange(
                        "(c p) u -> p c u", p=128))
                nc.sync.dma_start(
                    out=bi_col[:],
                    in_=b_i.ap().rearrange("(c p) -> p c", p=128))
                nc.sync.dma_start(
                    out=be_col[:],
                    in_=b_e.ap().rearrange("(c p) -> p c", p=128))
            if (mi, s) == (1, 1):
                nc.sync.dma_start(
                    out=w_e_sb[:],
                    in_=W_e.ap().bitcast(F32R).rearrange(
                        "(c p) u -> p c u", p=128))

        def emit_xp(mi, s):
            # transpose strip s into the psum pair tile; copy out on odd s
            m = MODS[mi]
            D = m["D"]
            xs = m["xs"].pop(s)
            half = (s % 2) * 256
            for dc in range(D // 128):
                if s % 2 == 0:
                    m["pst"][dc] = ps.tile([128, 512], F32R, tag="ps",
                                           name=f"pst_{mi}_{s}_{dc}")
                pst = m["pst"][dc]
                for tt in range(2):
                    nc.tensor.transpose(
                        pst[:, half + tt * 128:half + (tt + 1) * 128],
                        xs[:, tt, dc * 128:(dc + 1) * 128], identr)
            if s % 2 == 1:
                p0 = (s // 2) * 512
                for dc in range(D // 128):
                    nc.vector.tensor_copy(
                        m["tiles"][dc][:, p0:p0 + 512], m["pst"][dc][:])

        def emit_proj(mi, p):
            m = MODS[mi]
            D = m["D"]
            r0 = p * 512
            for uc in range(2):
                pp = psb.tile([128, 512], F32, tag="psb")
                for dc in range(D // 128):
                    nc.tensor.matmul(
                        pp[:], m["w"][:, dc, uc * 128:(uc + 1) * 128],
                        m["tiles"][dc][:, r0:r0 + 512],
                        start=(dc == 0), stop=(dc == D // 128 - 1))
                nc.scalar.add(m["projT"][:, uc, r0:r0 + 512], pp[:],
                              add=m["b"][:, uc:uc + 1])

        # PE warm-up: dummy transposes while the first strip DMA is in
        # flight, so the p-state ramp is done when real work arrives
        warm = ps.tile([128, 512], F32R, tag="ps", name="warm")
        for i in range(14):
            nc.tensor.transpose(warm[:, (i % 4) * 128:(i % 4 + 1) * 128],
                                identr[:], identr)

        # phase 1 pipeline over strip-pairs: image pairs 0..3, eeg pairs 0..3
        pairs = [(0, p) for p in range(NPAIR)] + \
                [(1, p) for p in range(NPAIR)]
        for k, (mi, p) in enumerate(pairs):
            emit_load(mi, 2 * p)
            emit_xp(mi, 2 * p)
            emit_load(mi, 2 * p + 1)
            emit_xp(mi, 2 * p + 1)
            if k > 0:
                emit_proj(*pairs[k - 1])
        emit_proj(*pairs[-1])

        # projT -> proj XBARs (SP queue)
        for uc in range(2):
            nc.sync.dma_start_transpose(proj_i[:, uc], projTi[:, uc, :])
        for uc in range(2):
            nc.sync.dma_start_transpose(proj_e[:, uc], projTe[:, uc, :])

        # ---- phase 2 ----
        ets = [None] * TQ
        finz = [None] * TQ

        def emit_scores(qt, av_qt=None):
            # score chunks for qt, with AV-eeg(av_qt) matmuls interleaved
            # between chunks so PSUM slot demand is smoothed against the
            # softmax latency chain
            cm = small.tile([128, 4], F32, tag="cm", name=f"cm_{qt}")
            s_chunks = []
            pav = None
            if av_qt is not None:
                ett = ets[av_qt]
                pav = psb.tile([128, 512], F32, tag="psb",
                               name=f"pav_{av_qt}")
            for nk in range(NK):
                s = ps.tile([128, 512], F32, tag="ps", name=f"s_{qt}_{nk}")
                s_chunks.append(s)
                for uc in range(2):
                    nc.tensor.matmul(
                        s[:],
                        projTe[:, uc, qt * 128:(qt + 1) * 128],
                        projTi[:, uc, nk * 512:(nk + 1) * 512],
                        start=(uc == 0), stop=(uc == 1))
                nc.vector.reduce_max(cm[:, nk:nk + 1], s[:], axis=AX)
                if pav is not None:
                    for j in range(4):
                        kc = nk * 4 + j
                        nc.tensor.matmul(
                            pav[:, :U], ett[:, kc, :], proj_e[:, :, kc, :],
                            start=(kc == 0), stop=(kc == TQ - 1))
            negmax = small.tile([128, 1], F32, tag="negmax", name=f"nm_{qt}")
            nc.vector.tensor_reduce(negmax[:], cm[:], axis=AX,
                                    op=mybir.AluOpType.max, negate=True)
            zp = small.tile([128, 4], F32, tag="zp", name=f"zp_{qt}")
            for nk in range(NK):
                nc.scalar.activation(
                    E[:, qt, nk * 512:(nk + 1) * 512], s_chunks[nk][:], EXP,
                    bias=negmax[:], scale=1.0, accum_out=zp[:, nk:nk + 1])
            ett2 = etp.tile([128, TQ, 128], F16, tag="ett", name=f"ett_{qt}")
            nc.sync.dma_start_transpose(ett2[:], E[:, qt, :])
            ets[qt] = ett2
            finz[qt] = zp
            if av_qt is not None:
                ets[av_qt] = None
                emit_av_out(av_qt, pav)

        def emit_finalize_z(qt):
            # on Pool (idle engine): Z row-sum and proj_i prescale;
            # reciprocal on DVE (no gpsimd reciprocal)
            zp = finz[qt]
            zh = small.tile([128, 2], F32, tag="zh", name=f"zh_{qt}")
            zrow = small.tile([128, 1], F32, tag="zrow", name=f"zr_{qt}")
            nc.gpsimd.tensor_add(zh[:], zp[:, 0:2], zp[:, 2:4])
            nc.gpsimd.tensor_add(zrow[:], zh[:, 0:1], zh[:, 1:2])
            nc.vector.reciprocal(rZ[:, qt:qt + 1], zrow[:])
            nc.gpsimd.tensor_scalar_mul(
                proj_i[:, :, qt, :], proj_i[:, :, qt, :], rZ[:, qt:qt + 1])

        oe_buf = [None]

        def emit_av_out(qt, pav):
            if qt % 2 == 0:
                oe_buf[0] = outp.tile([128, 2, U], F32, tag="oute",
                                      name=f"oe_{qt}")
            nc.scalar.activation(oe_buf[0][:, qt % 2, :], pav[:, :U], COPY,
                                 scale=rZ[:, qt:qt + 1])
            if qt % 2 == 1:
                q0 = (qt - 1) * 128
                nc.scalar.dma_start(
                    out=att_e.ap()[q0:q0 + 256, :].rearrange(
                        "(c p) u -> p c u", p=128),
                    in_=oe_buf[0][:])

        def emit_av_eeg(qt):
            ett = ets[qt]
            pav = psb.tile([128, 512], F32, tag="psb", name=f"pav_{qt}")
            for kc in range(TQ):
                nc.tensor.matmul(pav[:, :U], ett[:, kc, :],
                                 proj_e[:, :, kc, :],
                                 start=(kc == 0), stop=(kc == TQ - 1))
            ets[qt] = None
            emit_av_out(qt, pav)

        for qt in range(TQ):
            emit_scores(qt, av_qt=qt - AV_LAG if qt >= AV_LAG else None)
            if qt >= 1:
                emit_finalize_z(qt - 1)
        emit_finalize_z(TQ - 1)
        for qt in range(TQ - AV_LAG, TQ):
            emit_av_eeg(qt)

        # ---- phase 3: att_img[kt] = sum_q E[q, kt-block].T @ (proj_i/Z)[q]
        oi_buf = None
        for kt in range(TQ):
            pav = psb.tile([128, 512], F32, tag="psb", name=f"pvi_{kt}")
            for qc in range(TQ):
                nc.tensor.matmul(
                    pav[:, :U], E[:, qc, kt * 128:(kt + 1) * 128],
                    proj_i[:, :, qc, :],
                    start=(qc == 0), stop=(qc == TQ - 1))
            if kt % 2 == 0:
                oi_buf = outp.tile([128, 2, U], F32, tag="outi",
                                   name=f"oi_{kt}")
            nc.scalar.copy(oi_buf[:, kt % 2, :], pav[:, :U])
            if kt % 2 == 1:
                k0 = (kt - 1) * 128
                nc.scalar.dma_start(
                    out=att_i.ap()[k0:k0 + 256, :].rearrange(
                        "(c p) u -> p c u", p=128),
                    in_=oi_buf[:])

    nc.finalize()
    return nc


_NC_CACHE = {}


def kernel(eeg, image, W_e, b_e, W_i, b_i):
    key = "v4"
    if key not in _NC_CACHE:
        _NC_CACHE[key] = build()
    nc = _NC_CACHE[key]
    eeg = np.ascontiguousarray(eeg, dtype=np.float32)
    image = np.ascontiguousarray(image, dtype=np.float32)
    in_maps = [{
        "eeg": eeg[b], "image": image[b],
        "W_e": np.asarray(W_e, np.float32), "b_e": np.asarray(b_e, np.float32),
        "W_i": np.asarray(W_i, np.float32), "b_i": np.asarray(b_i, np.float32),
    } for b in range(B)]
    res = run_bass_kernel_spmd(nc, in_maps, list(range(NCORES)))
    att_e = np.stack([np.asarray(r["att_e"]) for r in res.results])
    att_i = np.stack([np.asarray(r["att_i"]) for r in res.results])
    return att_e, att_i
